# revision 14
# baseline (speedup 1.0000x reference)
"""Beta-TCVAE loss kernel for 8 Trainium2 NeuronCores.

Contract: kernel(**inputs) takes the FULL inputs (numpy), shards across
8 cores internally (data-parallel over batch; pairwise [B,B,L] tensor
sharded over the first batch axis), runs one SPMD Bass/Tile NEFF on
cores 0-7, and gathers to the full scalar loss.

Hardcoded problem shape: B=256, D=12288, L=32, f32.
"""

import numpy as np

import concourse.bacc as bacc
import concourse.bass_utils as bass_utils
import concourse.mybir as mybir
import concourse.tile as tile

N_CORES = 8
B, D, L = 256, 12288, 32
RPC = B // N_CORES          # 32 rows per core
P = 128                     # SBUF partitions
FBIG = RPC * D // P         # 3072 free elements per partition
NCHUNK = 4
W = FBIG // NCHUNK          # 768
NT = RPC * L // P           # 8 (i,l)-tiles of 128 partitions per core

DATASET_SIZE = 202599
BETA = 6.0
LOG2PI = float(np.log(2.0 * np.pi))
LOG_NM = float(np.log(float(B * DATASET_SIZE)))

F32 = mybir.dt.float32
AX = mybir.AxisListType
OP = mybir.AluOpType
AF = mybir.ActivationFunctionType

_STATE: dict = {}


def _build_nc(parts=("big", "qzx", "pair", "qz")):
    nc = bacc.Bacc("TRN2", target_bir_lowering=False, debug=False)

    # ---- per-core inputs ----
    tgt = nc.dram_tensor("tgt", [P, FBIG], F32, kind="ExternalInput").ap()
    xm = nc.dram_tensor("xm", [P, FBIG], F32, kind="ExternalInput").ap()
    xlv = nc.dram_tensor("xlv", [P, FBIG], F32, kind="ExternalInput").ap()
    zrow = nc.dram_tensor("zrow", [RPC, L], F32, kind="ExternalInput").ap()
    zmrow = nc.dram_tensor("zmrow", [RPC, L], F32, kind="ExternalInput").ap()
    zlvrow = nc.dram_tensor("zlvrow", [RPC, L], F32, kind="ExternalInput").ap()
    zT = nc.dram_tensor("zT", [L, RPC], F32, kind="ExternalInput").ap()
    zmT = nc.dram_tensor("zmT", [L, RPC], F32, kind="ExternalInput").ap()
    aT = nc.dram_tensor("aT", [L, B], F32, kind="ExternalInput").ap()
    zcol = nc.dram_tensor("zcol", [P, NT], F32, kind="ExternalInput").ap()
    zmcol = nc.dram_tensor("zmcol", [P, NT], F32, kind="ExternalInput").ap()

    # ---- per-core output, packed into one [128, 16] tensor ----
    # col 0        : q        (per-partition big-part partial: sum sq*e + sum lv)
    # col 3 [0:32] : qzp      (ln(sum exp) - 0.5*min  for log_qz)
    # col 4 [0:32] : s1       (sum_l dz^2 * exp(-zlv))
    # col 5 [0:32] : s2       (sum_l zlv)
    # col 6 [0:32] : s3       (sum_l z^2)
    # cols 8..16   : pcols    (ln sum_j exp(-0.5 M) per (i,l) partition)
    out_all = nc.dram_tensor("out_all", [P, 16], F32, kind="ExternalOutput").ap()

    with tile.TileContext(nc) as tc, \
            tc.tile_pool(name="big", bufs=3) as big, \
            tc.tile_pool(name="small", bufs=1) as small, \
            tc.tile_pool(name="ps", bufs=1, space="PSUM") as ps:

        res = small.tile([P, 16], F32)
        nc.vector.memset(res, 0.0)

        # ---- load small inputs (always, to keep the input set uniform) ----
        zrow_t = small.tile([RPC, L], F32)
        nc.sync.dma_start(out=zrow_t, in_=zrow)
        zmrow_t = small.tile([RPC, L], F32)
        nc.sync.dma_start(out=zmrow_t, in_=zmrow)
        zlvrow_t = small.tile([RPC, L], F32)
        nc.sync.dma_start(out=zlvrow_t, in_=zlvrow)
        zT_t = small.tile([L, RPC], F32)
        nc.sync.dma_start(out=zT_t, in_=zT)
        zmT_t = small.tile([L, RPC], F32)
        nc.sync.dma_start(out=zmT_t, in_=zmT)
        zcol_t = small.tile([P, NT], F32)
        nc.sync.dma_start(out=zcol_t, in_=zcol)
        zmcol_t = small.tile([P, NT], F32)
        nc.sync.dma_start(out=zmcol_t, in_=zmcol)

        # aT replicated 4x across partitions: aT_rep[p, j] = aT[p % 32, j]
        aT_rep = small.tile([P, B], F32)
        for r in range(P // L):
            nc.sync.dma_start(out=aT_rep[r * L:(r + 1) * L, :], in_=aT)
        eT_rep = small.tile([P, B], F32)
        nc.scalar.activation(out=eT_rep, in_=aT_rep, func=AF.Exp, scale=-1.0)

        if "pair" in parts:
            # --- log_qz_prod partials: logsumexp_j of M[i,j,l] per (i,l) ---
            # partition p of tile t <-> (i = 4t + p//32, l = p%32); free = j
            dcol = small.tile([P, NT], F32)
            nc.vector.tensor_sub(out=dcol, in0=zcol_t, in1=zmcol_t)
            d2col = small.tile([P, NT], F32)
            nc.vector.tensor_mul(out=d2col, in0=dcol, in1=dcol)

            Mbig = small.tile([P, NT, B], F32)
            for t in range(NT):
                # M'_t = d2col[:,t] * eT_rep + aT_rep   (fused DVE op)
                nc.vector.scalar_tensor_tensor(
                    out=Mbig[:, t, :], in0=eT_rep, scalar=d2col[:, t:t + 1],
                    in1=aT_rep, op0=OP.mult, op1=OP.add)
            Ebig = small.tile([P, NT, B], F32)
            nc.scalar.activation(out=Ebig, in_=Mbig, func=AF.Exp, scale=-0.5)
            smP = small.tile([P, NT], F32)
            nc.vector.tensor_reduce(out=smP, in_=Ebig, axis=AX.X, op=OP.add)
            nc.scalar.activation(out=res[:, 8:16], in_=smP, func=AF.Ln)

        if "qz" in parts:
            # --- log_qz partials: logsumexp_j of S[i,j] = c2 - 0.5*H[i,j] ---
            # H = (z-zm)^2(T) @ eT + ones @ aT  via two accumulating matmuls
            dT = small.tile([L, RPC], F32)
            nc.vector.tensor_sub(out=dT, in0=zT_t, in1=zmT_t)
            dT2 = small.tile([L, RPC], F32)
            nc.vector.tensor_mul(out=dT2, in0=dT, in1=dT)
            ones = small.tile([L, RPC], F32)
            nc.vector.memset(ones, 1.0)

            H = ps.tile([RPC, B], F32)
            nc.tensor.matmul(H[:, :], dT2[:, :], eT_rep[0:L, :],
                             start=True, stop=False)
            nc.tensor.matmul(H[:, :], ones[:, :], aT_rep[0:L, :],
                             start=False, stop=True)

            mn = small.tile([RPC, 1], F32)
            nc.vector.tensor_reduce(out=mn, in_=H[:, :], axis=AX.X, op=OP.min)
            mnh = small.tile([RPC, 1], F32)
            nc.vector.tensor_scalar_mul(out=mnh, in0=mn, scalar1=0.5)
            Ejunk = small.tile([RPC, B], F32)
            smq = small.tile([RPC, 1], F32)
            nc.scalar.activation(out=Ejunk, in_=H[:, :], func=AF.Exp,
                                 scale=-0.5, bias=mnh, accum_out=smq)
            lnsm = small.tile([RPC, 1], F32)
            nc.scalar.activation(out=lnsm, in_=smq, func=AF.Ln)
            # qzp = -0.5*mn + ln(smq)
            nc.vector.scalar_tensor_tensor(
                out=res[0:RPC, 3:4], in0=mn, scalar=-0.5, in1=lnsm,
                op0=OP.mult, op1=OP.add)

        if "qzx" in parts:
            # --- log_qzx / log_pz row partials ---
            e2 = small.tile([RPC, L], F32)
            nc.scalar.activation(out=e2, in_=zlvrow_t, func=AF.Exp, scale=-1.0)
            dz = small.tile([RPC, L], F32)
            nc.vector.tensor_sub(out=dz, in0=zrow_t, in1=zmrow_t)
            dz2 = small.tile([RPC, L], F32)
            nc.vector.tensor_mul(out=dz2, in0=dz, in1=dz)
            junkA = small.tile([RPC, L], F32)
            nc.vector.scalar_tensor_tensor(
                out=junkA, in0=dz2, scalar=1.0, in1=e2,
                op0=OP.mult, op1=OP.mult, accum_out=res[0:RPC, 4:5])
            junkB = small.tile([RPC, L], F32)
            nc.vector.tensor_scalar(
                out=junkB, in0=zlvrow_t, scalar1=0.0, scalar2=None,
                op0=OP.add, op1=OP.add, accum_out=res[0:RPC, 5:6])
            junkC = small.tile([RPC, L], F32)
            nc.vector.scalar_tensor_tensor(
                out=junkC, in0=zrow_t, scalar=1.0, in1=zrow_t,
                op0=OP.mult, op1=OP.mult, accum_out=res[0:RPC, 6:7])

        subs = {p.split(":")[1] for p in parts if p.startswith("big:")}
        if "big" in parts:
            subs = {"exp", "sub", "sq", "stt", "lv"}
        if subs:
            # ================= big log_px part =================
            stats = small.tile([P, 2 * NCHUNK], F32)
            nc.vector.memset(stats, 0.0)
            for k in range(NCHUNK):
                sl = slice(k * W, (k + 1) * W)
                t_t = big.tile([P, W], F32)
                nc.sync.dma_start(out=t_t, in_=tgt[:, sl])
                m_t = big.tile([P, W], F32)
                nc.sync.dma_start(out=m_t, in_=xm[:, sl])
                v_t = big.tile([P, W], F32)
                nc.sync.dma_start(out=v_t, in_=xlv[:, sl])

                e_t = big.tile([P, W], F32)
                if "exp" in subs:
                    nc.scalar.activation(out=e_t, in_=v_t, func=AF.Exp,
                                         scale=-1.0)
                else:
                    nc.vector.tensor_copy(out=e_t, in_=v_t)
                d_t = big.tile([P, W], F32)
                if "sub" in subs:
                    nc.vector.tensor_sub(out=d_t, in0=t_t, in1=m_t)
                else:
                    nc.vector.tensor_copy(out=d_t, in_=t_t)
                sq_t = big.tile([P, W], F32)
                if "sq" in subs:
                    nc.scalar.activation(out=sq_t, in_=d_t, func=AF.Square)
                else:
                    nc.vector.tensor_copy(out=sq_t, in_=d_t)
                if "stt" in subs:
                    junk = big.tile([P, W], F32)
                    nc.vector.scalar_tensor_tensor(
                        out=junk, in0=sq_t, scalar=1.0, in1=e_t,
                        op0=OP.mult, op1=OP.mult, accum_out=stats[:, k:k + 1])
                elif "mulred" in subs:
                    junk = big.tile([P, W], F32)
                    nc.vector.tensor_mul(out=junk, in0=sq_t, in1=e_t)
                    nc.vector.reduce_sum(out=stats[:, k:k + 1], in_=junk,
                                         axis=AX.X)
                elif "ttr" in subs:
                    junk = big.tile([P, W], F32)
                    nc.vector.tensor_tensor_reduce(
                        out=junk, in0=sq_t, in1=e_t, scale=1.0, scalar=0.0,
                        op0=OP.mult, op1=OP.add, accum_out=stats[:, k:k + 1])
                else:
                    nc.vector.reduce_sum(out=stats[:, k:k + 1], in_=sq_t,
                                         axis=AX.X)
                if "lv" in subs:
                    junk2 = big.tile([P, W], F32)
                    nc.vector.tensor_scalar(
                        out=junk2, in0=v_t, scalar1=0.0, scalar2=None,
                        op0=OP.add, op1=OP.add,
                        accum_out=stats[:, NCHUNK + k:NCHUNK + k + 1])
                else:
                    nc.vector.reduce_sum(
                        out=stats[:, NCHUNK + k:NCHUNK + k + 1], in_=e_t,
                        axis=AX.X)
            nc.vector.reduce_sum(out=res[:, 0:1], in_=stats, axis=AX.X)

        nc.sync.dma_start(out=out_all, in_=res)

    nc.compile()
    return nc


def _shard_inputs(target, x_mean, x_log_var, z, z_mean, z_log_var):
    f32 = np.float32
    target = np.ascontiguousarray(target, dtype=f32)
    x_mean = np.ascontiguousarray(x_mean, dtype=f32)
    x_log_var = np.ascontiguousarray(x_log_var, dtype=f32)
    z = np.ascontiguousarray(z, dtype=f32)
    z_mean = np.ascontiguousarray(z_mean, dtype=f32)
    z_log_var = np.ascontiguousarray(z_log_var, dtype=f32)

    aT = np.ascontiguousarray(z_log_var.T)  # [L, B]
    in_maps = []
    for c in range(N_CORES):
        rows = slice(c * RPC, (c + 1) * RPC)
        z_sh = z[rows]
        zm_sh = z_mean[rows]
        in_maps.append({
            "tgt": np.ascontiguousarray(target[rows]).reshape(P, FBIG),
            "xm": np.ascontiguousarray(x_mean[rows]).reshape(P, FBIG),
            "xlv": np.ascontiguousarray(x_log_var[rows]).reshape(P, FBIG),
            "zrow": np.ascontiguousarray(z_sh),
            "zmrow": np.ascontiguousarray(zm_sh),
            "zlvrow": np.ascontiguousarray(z_log_var[rows]),
            "zT": np.ascontiguousarray(z_sh.T),
            "zmT": np.ascontiguousarray(zm_sh.T),
            "aT": aT,
            "zcol": np.ascontiguousarray(z_sh.reshape(NT, P).T),
            "zmcol": np.ascontiguousarray(zm_sh.reshape(NT, P).T),
        })
    return in_maps


def _gather(results) -> np.float32:
    """Combine the 8 per-core [128,16] outputs into the scalar loss."""
    v_all = np.empty((B,), dtype=np.float64)
    c3 = -0.5 * LOG2PI
    c2 = -0.5 * L * LOG2PI
    for c, r in enumerate(results):
        o = np.asarray(r["out_all"], dtype=np.float64)
        q = o[:, 0]
        qzp = o[0:RPC, 3]
        s1 = o[0:RPC, 4]
        s2 = o[0:RPC, 5]
        s3 = o[0:RPC, 6]
        pcols = o[:, 8:16]

        log_px = -0.5 * (D * LOG2PI + q.reshape(RPC, 4).sum(axis=1))
        log_qzx = -0.5 * (L * LOG2PI + s2 + s1)
        log_pz = -0.5 * (L * LOG2PI + s3)
        log_qz = c2 + qzp - LOG_NM

        # pcols[p, t] = ln sum_j exp(-0.5 M') for (i = 4t + p//32, l = p%32)
        p_sum = np.empty((RPC,), dtype=np.float64)
        for t in range(NT):
            col = pcols[:, t].reshape(4, L)  # rows r -> i = 4t + r
            p_sum[4 * t:4 * t + 4] = col.sum(axis=1)
        log_qz_prod = L * c3 + p_sum - L * LOG_NM

        v = (log_px - log_qzx + (1.0 - BETA) * (log_qz - log_qz_prod)
             + log_pz)
        v_all[c * RPC:(c + 1) * RPC] = v
    return np.float32(-v_all.mean())


def kernel(target, x_mean, x_log_var, z, z_mean, z_log_var) -> np.ndarray:
    if "nc" not in _STATE:
        _STATE["nc"] = _build_nc()
    nc = _STATE["nc"]
    in_maps = _shard_inputs(target, x_mean, x_log_var, z, z_mean, z_log_var)
    res = bass_utils.run_bass_kernel_spmd(nc, in_maps, core_ids=list(range(N_CORES)))
    return np.asarray(_gather(res.results))
